# revision 22
# baseline (speedup 1.0000x reference)
"""Trainium2 Bass kernel for nn_AttentionMemory (sparse_attention).

reference:
    mkf = mk.reshape(B, CK, HW); qkf = qk.reshape(B, CK, HW)
    affinity[b, m, q] = (-|mk_m|^2 + 2 mk_m.qk_q - |qk_q|^2) / sqrt(CK)
    out = softmax(affinity, axis=m)

Math used here: softmax over m is invariant to any additive term that is
constant in m, so the -|qk_q|^2 term (and any global constant C) drops out:
    out[b, :, q] = softmax_m( mk_m.qk_q/4 - (|mk_m|^2 - C)/8 )
with sqrt(CK) = 8, C = 64 (centers the exponent near 0).

Distribution: pure data parallelism over B=16 -> 2 batches per core on 8
NeuronCores. Each core computes out[b, :, :] for its 2 batches.

Per-core layout (orientation: m on partitions, q on free axis so the output
DMA is contiguous):
  - lhsT (stationary) = mkaug[65, 2304]: rows 0..63 = mk/4, row 64 = (a-64)/8
    where a[m] = sum_c mk[c,m]^2.
  - rhs (moving)      = qkaug[65, 2304]: rows 0..63 = qk, row 64 = -1.
  - matmul (fp32r, 1 cyc/row) -> PSUM z[m_tile, q_chunk]
  - ScalarE exp -> E in SBUF
  - column sums via all-ones[128,128] matmul accumulated over the 18 m-tiles
    (result is s[q] broadcast to every partition for free)
  - reciprocal_approx_fast, then VectorE in-place multiply, DMA out.
"""

import numpy as np
from contextlib import ExitStack

import concourse.bass as bass
import concourse.tile as tile
from concourse import bacc, mybir
from concourse.bass_utils import run_bass_kernel_spmd

B, CK, H, W = 16, 64, 36, 64
HW = H * W                 # 2304
NCORES = 8
BL = B // NCORES           # 2 batches per core
MT = HW // 128             # 18 m-tiles
QCH = 768                  # q chunk (2 PSUM banks)
NQ = HW // QCH             # 3
F32 = mybir.dt.float32
F32R = mybir.dt.float32r
AF = mybir.ActivationFunctionType
ALU = mybir.AluOpType

# matmul free-dim chunks inside one 768-wide q chunk (PSUM bank = 512 f32)
MM_CHUNKS = ((0, 512), (512, 256))


def _build_kernel(tc: tile.TileContext, out_ext, mk_ext, qk_ext):
    nc = tc.nc
    ones_dram = nc.inline_tensor(np.ones((128, HW), np.float32))
    with ExitStack() as ctx:
        singles = ctx.enter_context(tc.tile_pool(name="singles", bufs=1))
        mkaug = [singles.tile([CK + 1, HW], F32R, name=f"mkaug{b}") for b in range(BL)]
        qkaug = [singles.tile([CK + 1, HW], F32R, name=f"qkaug{b}") for b in range(BL)]
        ones_sb = singles.tile([128, HW], F32, name="ones_sb")
        nc.sync.dma_start(ones_sb[:], ones_dram.ap())
        ones_r = singles.tile([128, 128], F32R, name="ones_r")
        nc.gpsimd.tensor_copy(ones_r[:], ones_sb[:, 0:128])
        # tiny dummy exp so the ACT table set loads at t~0, not before the
        # first real Activation
        dummy = singles.tile([1, 1], F32, name="dummy")
        nc.scalar.activation(dummy[:], ones_sb[0:1, 0:1], AF.Exp)

        prep = ctx.enter_context(tc.tile_pool(name="prep", bufs=1))
        e_sb_pool = ctx.enter_context(tc.tile_pool(name="e_sb", bufs=2))
        o_pool = ctx.enter_context(tc.tile_pool(name="o_sb", bufs=7))
        r_pool = ctx.enter_context(tc.tile_pool(name="r", bufs=2))
        psum_e = ctx.enter_context(tc.tile_pool(name="psum_e", bufs=2, space="PSUM"))
        psum_s = ctx.enter_context(tc.tile_pool(name="psum_s", bufs=2, space="PSUM"))

        def emit_prep(b):
            # mkaug rows 0..63 = mk/4 (rounded to f32r); qkaug rows 0..63 =
            # qk; sq16 = mk^2/16 (f32r); a16 = sum_c sq16; since qkaug row 64
            # is +1, mkaug row 64 = -(a - 64)/8 = -2*a16 + 8.
            mk_raw = prep.tile([CK, HW], F32, name="mk_raw")
            qk_raw = prep.tile([CK, HW], F32, name="qk_raw")
            nc.sync.dma_start(mk_raw[:], mk_ext[b])
            nc.sync.dma_start(qk_raw[:], qk_ext[b])
            nc.gpsimd.tensor_copy(qkaug[b][0:CK, :], qk_raw[:])
            nc.gpsimd.tensor_copy(qkaug[b][CK : CK + 1, :], ones_sb[0:1, :])
            nc.vector.tensor_scalar_mul(mkaug[b][0:CK, :], mk_raw[:], 0.25)
            sq = prep.tile([CK, HW], F32R, name="sq")
            nc.vector.scalar_tensor_tensor(
                out=sq[:],
                in0=mk_raw[:],
                scalar=0.0625,
                in1=mk_raw[:],
                op0=ALU.mult,
                op1=ALU.mult,
            )
            for seg in range(NQ):
                a_ps = psum_e.tile([128, QCH], F32, name="e_ps")[0:1, :]
                for off, size in MM_CHUNKS:
                    nc.tensor.matmul(
                        a_ps[0:1, off : off + size],
                        lhsT=ones_r[0:CK, 0:1],
                        rhs=sq[:, seg * QCH + off : seg * QCH + off + size],
                        start=True,
                        stop=True,
                    )
                nc.vector.tensor_scalar(
                    out=mkaug[b][CK : CK + 1, seg * QCH : (seg + 1) * QCH],
                    in0=a_ps[:],
                    scalar1=-2.0,
                    scalar2=8.0,
                    op0=ALU.mult,
                    op1=ALU.add,
                )

        emit_prep(0)

        for job, (b, qi) in enumerate([(b, qi) for b in range(BL) for qi in range(NQ)]):
            if job == 1:
                # b=1 prep is emitted after the first chunk so it doesn't
                # delay the first output on the in-order engines
                emit_prep(1)
            if True:
                q0 = qi * QCH
                e_sb = e_sb_pool.tile([128, MT, QCH], F32R, name="e_sb")
                s_ps = psum_s.tile([128, QCH], F32, name="s_ps")

                def sum_mm(t):
                    # s[q] (broadcast across all partitions) via all-ones lhsT,
                    # accumulated over the 18 m-tiles
                    for off, size in MM_CHUNKS:
                        nc.tensor.matmul(
                            s_ps[:, off : off + size],
                            lhsT=ones_r[:],
                            rhs=e_sb[:, t, off : off + size],
                            start=(t == 0),
                            stop=(t == MT - 1),
                        )

                # z = mkaug.T @ qkaug -> exp -> SBUF; the sum-matmul for tile
                # t-1 is interleaved after the main matmul of tile t so the
                # in-order TensorE pipeline never waits a full exp phase.
                for t in range(MT):
                    e_ps = psum_e.tile([128, QCH], F32, name="e_ps")
                    for off, size in MM_CHUNKS:
                        nc.tensor.matmul(
                            e_ps[:, off : off + size],
                            lhsT=mkaug[b][:, t * 128 : (t + 1) * 128],
                            rhs=qkaug[b][:, q0 + off : q0 + off + size],
                            start=True,
                            stop=True,
                        )
                    nc.scalar.activation(e_sb[:, t, :], e_ps[:], AF.Exp)
                    if t >= 1:
                        sum_mm(t - 1)
                sum_mm(MT - 1)
                r_sb = r_pool.tile([128, QCH], F32, name="r_sb")
                nc.vector.reciprocal_approx_fast(r_sb[:], s_ps[:])
                # normalize and store
                for t in range(MT):
                    o_sb = o_pool.tile([128, QCH], F32, name="o_sb")
                    nc.vector.tensor_mul(o_sb[:], e_sb[:, t, :].bitcast(F32), r_sb[:])
                    nc.sync.dma_start(
                        out_ext[b, t * 128 : (t + 1) * 128, q0 : q0 + QCH], o_sb[:]
                    )


_CACHE = {}


def _get_compiled(niter: int = 1):
    """Build+compile the per-core graph. niter>1 repeats the whole body
    (same inputs/outputs) for wall-clock-difference timing in test.py."""
    key = ("nc", niter)
    if key not in _CACHE:
        nc = bacc.Bacc("TRN2", target_bir_lowering=False, debug=False)
        mk_ext = nc.dram_tensor("mk", [BL, CK, HW], F32, kind="ExternalInput").ap()
        qk_ext = nc.dram_tensor("qk", [BL, CK, HW], F32, kind="ExternalInput").ap()
        out_ext = nc.dram_tensor("out", [BL, HW, HW], F32, kind="ExternalOutput").ap()
        with tile.TileContext(nc) as tc:
            for _ in range(niter):
                _build_kernel(tc, out_ext, mk_ext, qk_ext)
        nc.compile()
        _CACHE[key] = nc
    return _CACHE[key]


def run_spmd(mk: np.ndarray, qk: np.ndarray, niter: int = 1) -> np.ndarray:
    mk = np.ascontiguousarray(np.asarray(mk, dtype=np.float32).reshape(B, CK, HW))
    qk = np.ascontiguousarray(np.asarray(qk, dtype=np.float32).reshape(B, CK, HW))
    nc = _get_compiled(niter)
    in_maps = [
        {"mk": mk[c * BL : (c + 1) * BL], "qk": qk[c * BL : (c + 1) * BL]}
        for c in range(NCORES)
    ]
    res = run_bass_kernel_spmd(nc, in_maps, core_ids=list(range(NCORES)))
    out = np.concatenate([res.results[c]["out"] for c in range(NCORES)], axis=0)
    return out.reshape(B, HW, HW)


def kernel(mk: np.ndarray, qk: np.ndarray) -> np.ndarray:
    return run_spmd(mk, qk, niter=1)
